# revision 7
# baseline (speedup 1.0000x reference)
"""Multi-head causal attention (B=2, T=2048, C=1024, H=16) on 8 trn2 NeuronCores.

Sharding: 2 heads per core (tensor-parallel over heads), both batch elements
on every core. Key performance structure (v2):

  * Per-core batch-order permutation: core c computes the OTHER half's batch
    first (slot 0), its own half's batch second (slot 1). Attention-output
    exchange runs as chunked shared-output AllGathers: one per 512-token
    chunk, fired as soon as that chunk is done, so the early exchange hides
    under slot-0 attention and most of the late exchange hides under slot-1.
    Each core then reads exactly the 8 sender-chunks it needs (its own token
    block) from the shared buffers with partition-id-based dynamic DMA
    offsets, keeping the program SPMD-uniform with a minimal 8-step output
    projection.
  * The scalar engine's exp (153.6 Ge/s peak) cannot feed the PE at 2.4 GHz,
    and any PE idle gap resets the p-state ramp (1.2 GHz until 3us of
    continuous execution). So the attention inner loop is padded with
    independent matmul work: slot-1's qkv projection is interleaved into
    slot-0's attention, and dummy matmuls pad slot-1's attention.
  * Attention chunks run in reversed order (largest causal row first) so the
    per-chunk AllGathers fire as early as possible.
  * bf16 operands everywhere (same PE rate as fp16, lower power -> less HAM
    duty-cycle throttling), fp32 PSUM accumulation.
  * x is DMA'd in per-k-tile chunks so the first qkv matmul starts early;
    warm-up matmuls on locally-generated tiles ramp the PE p-state under the
    DMA; a tiny warm-up AllGather pre-opens the collective channel.

Host side shards/permutes/casts inputs (bf16) and reassembles the output.
"""

import sys

import numpy as np

if "/opt/trn_rl_repo" not in sys.path:
    sys.path.insert(0, "/opt/trn_rl_repo")

B, T, C, H, D = 2, 2048, 1024, 16, 64
NCORES = 8
HPC = H // NCORES          # heads per core = 2
CW = HPC * D               # per-core channel width = 128
KT = C // 128              # k tiles = 8
TT = T // 128              # t tiles = 16
SHARD = (B * T) // NCORES  # output rows per core = 512
SCALE = 1.0 / float(np.sqrt(C))

_CACHE = {}
LAST_EXEC_NS = None


def _build_nc():
    import concourse.mybir as mybir
    import concourse.tile as tile
    from concourse import bacc
    from concourse.masks import make_identity, make_upper_triangular

    f32 = mybir.dt.float32
    bf16 = mybir.dt.bfloat16

    nc = bacc.Bacc("TRN2", target_bir_lowering=False, debug=False,
                   num_devices=NCORES)

    xT = nc.dram_tensor("xT", [2, 128, KT * T], bf16, kind="ExternalInput")
    wq = nc.dram_tensor("wq", [128, KT * CW], bf16, kind="ExternalInput")
    wk = nc.dram_tensor("wk", [128, KT * CW], bf16, kind="ExternalInput")
    wv = nc.dram_tensor("wv", [128, KT * CW], bf16, kind="ExternalInput")
    wpE = nc.dram_tensor("wpE", [128, 4 * C], bf16, kind="ExternalInput")
    wpL = nc.dram_tensor("wpL", [128, 4 * C], bf16, kind="ExternalInput")
    bq = nc.dram_tensor("bq", [CW, 1], f32, kind="ExternalInput")
    bk = nc.dram_tensor("bk", [CW, 1], f32, kind="ExternalInput")
    bv = nc.dram_tensor("bv", [CW, 1], f32, kind="ExternalInput")
    bp = nc.dram_tensor("bp", [1, C], f32, kind="ExternalInput")
    y = nc.dram_tensor("y", [SHARD, C], f32, kind="ExternalOutput")

    agE_in = nc.dram_tensor("agE_in", [4, 128, 512], bf16)
    agE_out = nc.dram_tensor("agE_out", [NCORES, 4, 128, 512], bf16,
                             addr_space="Shared")
    agL_in = nc.dram_tensor("agL_in", [4, 128, 512], bf16)
    agL_out = nc.dram_tensor("agL_out", [NCORES, 4, 128, 512], bf16,
                             addr_space="Shared")
    warm_i = nc.dram_tensor("warm_i", [8, 16], f32)
    warm_o = nc.dram_tensor("warm_o", [NCORES, 8, 16], f32)

    with tile.TileContext(nc) as tc:
        with (
            tc.tile_pool(name="const", bufs=1) as const,
            tc.tile_pool(name="xtp", bufs=1) as xtp,
            tc.tile_pool(name="wqkv", bufs=1) as wqkvp,
            tc.tile_pool(name="qkv", bufs=1) as qkvp,
            tc.tile_pool(name="pt", bufs=3) as ptp,
            tc.tile_pool(name="otp", bufs=1) as otp,
            tc.tile_pool(name="sm", bufs=1) as smp,
            tc.tile_pool(name="proj", bufs=1) as projp,
            tc.tile_pool(name="ysb", bufs=2) as ysbp,
        ):
            # ---- x DMA, split per k-tile so the first matmul starts early --
            xt = {}
            for s in range(2):
                xt[s] = [xtp.tile([128, T], bf16, name=f"xt{s}_{a}")
                         for a in range(KT)]
            for s in range(2):
                for a in range(KT):
                    nc.sync.dma_start(xt[s][a][:], xT[s, :, T * a : T * (a + 1)])

            # ---- collective warm-up (channel init overlaps compute) --------
            wtile = const.tile([8, 16], f32, name="wtile")
            nc.vector.memset(wtile[:], 0.0)
            nc.sync.dma_start(warm_i[:], wtile[:])
            nc.gpsimd.collective_compute(
                "AllGather", mybir.AluOpType.bypass,
                replica_groups=[list(range(NCORES))],
                ins=[warm_i[:].opt()], outs=[warm_o[:].opt()],
            )

            # ---- constants -------------------------------------------------
            trimask = const.tile([128, 128], bf16, name="trimask")
            make_upper_triangular(nc, trimask[:], val=1.0, diag=True)
            ident = const.tile([128, 128], bf16, name="ident")
            make_identity(nc, ident[:])
            wscr = const.tile([128, 512], bf16, name="wscr")
            nc.vector.memset(wscr[:], 0.0)

            bq_t = const.tile([CW, 1], f32, name="bq_t")
            bk_t = const.tile([CW, 1], f32, name="bk_t")
            bv_t = const.tile([CW, 1], f32, name="bv_t")
            nc.sync.dma_start(bq_t[:], bq[:])
            nc.sync.dma_start(bk_t[:], bk[:])
            nc.sync.dma_start(bv_t[:], bv[:])
            bp_row = const.tile([1, C], f32, name="bp_row")
            nc.sync.dma_start(bp_row[:], bp[:])
            bpb = const.tile([128, C], f32, name="bpb")
            nc.gpsimd.partition_broadcast(bpb[:], bp_row[:])

            # ---- weights ---------------------------------------------------
            wq_sb = wqkvp.tile([128, KT * CW], bf16, name="wq_sb")
            wk_sb = wqkvp.tile([128, KT * CW], bf16, name="wk_sb")
            wv_sb = wqkvp.tile([128, KT * CW], bf16, name="wv_sb")
            nc.sync.dma_start(wq_sb[:], wq[:])
            nc.sync.dma_start(wk_sb[:], wk[:])
            nc.sync.dma_start(wv_sb[:], wv[:])
            wpE_sb = projp.tile([128, 4 * C], bf16, name="wpE_sb")
            wpL_sb = projp.tile([128, 4 * C], bf16, name="wpL_sb")
            nc.sync.dma_start(wpE_sb[:], wpE[:])
            nc.sync.dma_start(wpL_sb[:], wpL[:])

            # ---- psum pools ------------------------------------------------
            qkv_psum = tc.tile_pool(name="psqk", bufs=2, space="PSUM")
            psqk = qkv_psum.__enter__()
            attn_psum_s = tc.tile_pool(name="ps_s", bufs=2, space="PSUM")
            ps_s = attn_psum_s.__enter__()
            attn_psum_o = tc.tile_pool(name="ps_o", bufs=1, space="PSUM")
            ps_o = attn_psum_o.__enter__()

            # ---- PE p-state ramp: warm-ups on local tiles (no DMA gate) ----
            warm_ps = psqk.tile([128, 512], f32, name="warm_ps", tag="ps_qk")
            for _ in range(36):
                nc.tensor.matmul(
                    warm_ps[:], trimask[:], wscr[:], start=True, stop=True,
                )
            nc.vector.memset(warm_ps[:, 0:2], 0.0)

            qT_sb, kT_sb, v_sb, ot_sb, r_all = {}, {}, {}, {}, {}

            # per-core rank values (gpsimd registers) for dynamic addressing
            pid = nc.gpsimd.partition_id()
            half = pid // 4
            opp = (half + 1) % 2
            myj = pid % 4

            def qkv_steps(s):
                """Generator: emits qkv projection for slot s, yielding after
                each PE instruction so it can interleave with attention."""
                qT_b = qkvp.tile([128, T], bf16, name=f"qT{s}")
                kT_b = qkvp.tile([128, T], bf16, name=f"kT{s}")
                qT_sb[s], kT_sb[s] = qT_b, kT_b
                for dst, w_sb, bias in ((qT_b, wq_sb, bq_t), (kT_b, wk_sb, bk_t)):
                    for j in range(4):
                        ps = psqk.tile([128, 512], f32, name="ps_qk", tag="ps_qk")
                        for a in range(KT):
                            nc.tensor.matmul(
                                ps[:],
                                w_sb[:, CW * a : CW * (a + 1)],
                                xt[s][a][:, 512 * j : 512 * (j + 1)],
                                start=(a == 0), stop=(a == KT - 1),
                            )
                            yield
                        nc.vector.tensor_scalar_add(
                            dst[:, 512 * j : 512 * (j + 1)], ps[:], bias[:]
                        )
                vT_b = qkvp.tile([128, T], bf16, name=f"vT{s}")
                for j in range(4):
                    ps = psqk.tile([128, 512], f32, name="ps_vT", tag="ps_qk")
                    for a in range(KT):
                        nc.tensor.matmul(
                            ps[:],
                            wv_sb[:, CW * a : CW * (a + 1)],
                            xt[s][a][:, 512 * j : 512 * (j + 1)],
                            start=(a == 0), stop=(a == KT - 1),
                        )
                        yield
                    nc.vector.tensor_scalar_add(
                        vT_b[:, 512 * j : 512 * (j + 1)], ps[:], bv_t[:]
                    )
                v_b = []
                for m in range(TT):
                    vt = qkvp.tile([128, 2 * (D + 1)], bf16, name=f"v{s}_{m}")
                    tps = psqk.tile([128, 128], bf16, name="ps_tr", tag="ps_qk")
                    nc.tensor.transpose(
                        tps[:], vT_b[:, 128 * m : 128 * (m + 1)], ident[:]
                    )
                    yield
                    nc.vector.tensor_copy(
                        vt[:].rearrange("p (a m) -> p a m", a=2)[:, :, 0:D],
                        tps[:].rearrange("p (a m) -> p a m", a=2),
                    )
                    nc.vector.memset(vt[:, D : D + 1], 1.0)
                    nc.vector.memset(vt[:, 2 * D + 1 : 2 * D + 2], 1.0)
                    v_b.append(vt)
                v_sb[s] = v_b

            def filler_dummy():
                """Endless PE keep-busy matmuls into a cycled psum tile."""
                while True:
                    ps = psqk.tile([128, 512], f32, name="ps_dmy", tag="ps_qk")
                    nc.tensor.matmul(
                        ps[:], trimask[:], wscr[:], start=True, stop=True,
                    )
                    yield

            def emit_attn(s, filler, fill_per_step, post_chunk=None):
                ot = otp.tile([128, T], bf16, name=f"ot{s}")
                ot_sb[s] = ot
                ra = smp.tile([1, 4096], f32, name=f"r_all{s}", tag="r_all")
                r_all[s] = ra
                ag_in = agE_in if s == 0 else agL_in
                for j in (3, 2, 1, 0):
                    o_ps = [
                        ps_o.tile([65, 512], f32, name=f"o{h}", tag=f"o{h}")
                        for h in range(2)
                    ]
                    ilast = 4 * (j + 1) - 1
                    for i in range(4 * (j + 1)):
                        off = max(0, 128 * i - 512 * j)
                        s_ps = ps_s.tile([128, 1024], f32, name="s_ps", tag="s")
                        pt = ptp.tile([128, 1024], bf16, name="pt", tag="pt")
                        for h in range(2):
                            nc.tensor.matmul(
                                s_ps[:, 512 * h + off : 512 * (h + 1)],
                                kT_sb[s][64 * h : 64 * h + 64,
                                         128 * i : 128 * (i + 1)],
                                qT_sb[s][64 * h : 64 * h + 64,
                                         512 * j + off : 512 * (j + 1)],
                                start=True, stop=True,
                            )
                        # PE filler while ScalarE computes the exp
                        for _ in range(fill_per_step):
                            next(filler, None)
                        nc.scalar.activation(
                            pt[:].rearrange("p (g w) -> p g w", g=2)[:, :, off:512],
                            s_ps[:].rearrange("p (g w) -> p g w", g=2)[:, :, off:512],
                            mybir.ActivationFunctionType.Exp,
                            scale=SCALE,
                        )
                        if 4 * j <= i:
                            for h in range(2):
                                nc.gpsimd.tensor_tensor(
                                    pt[:, 512 * h + off : 512 * h + off + 128],
                                    pt[:, 512 * h + off : 512 * h + off + 128],
                                    trimask[:],
                                    op=mybir.AluOpType.mult,
                                )
                        for h in range(2):
                            nc.tensor.matmul(
                                o_ps[h][0:65, off:512],
                                v_sb[s][i][:, (D + 1) * h : (D + 1) * (h + 1)],
                                pt[:, 512 * h + off : 512 * (h + 1)],
                                start=(i == 0), stop=(i == ilast),
                            )
                    # rowsums, reciprocal, fused divide+evict for chunk j
                    for h in range(2):
                        idx = 2 * j + h
                        nc.vector.tensor_copy(
                            ra[0:1, 512 * idx : 512 * (idx + 1)],
                            o_ps[h][64:65, :],
                        )
                    rs = ra[0:1, 1024 * j : 1024 * j + 1024]
                    nc.vector.reciprocal_approx_fast(rs, rs)
                    rb = smp.tile([128, 1024], f32, name="rb", tag="rb", bufs=2)
                    nc.gpsimd.partition_broadcast(rb[:], rs)
                    for h in range(2):
                        nc.vector.tensor_tensor(
                            ot[64 * h : 64 * h + 64, 512 * j : 512 * (j + 1)],
                            o_ps[h][0:64, :],
                            rb[64 * h : 64 * h + 64, 512 * h : 512 * (h + 1)],
                            op=mybir.AluOpType.mult,
                        )
                    nc.sync.dma_start(ag_in[j],
                                       ot[:, 512 * j : 512 * (j + 1)])
                    if post_chunk is not None:
                        post_chunk(j)

            # slot-0 qkv runs standalone (PE-dense)
            for _ in qkv_steps(0):
                pass
            # slot-0 attention interleaves slot-1 qkv as PE filler
            g1 = qkv_steps(1)
            emit_attn(0, g1, 3)
            for _ in g1:  # finish any qkv(1) remainder
                pass

            # early exchange: one AllGather of this core's slot-0 output
            nc.gpsimd.collective_compute(
                "AllGather", mybir.AluOpType.bypass,
                replica_groups=[list(range(NCORES))],
                ins=[agE_in[:].opt()], outs=[agE_out[:].opt()],
            )

            yTE = [projp.tile([128, 512], bf16, name=f"yTE{i}")
                   for i in range(4)]
            yTL = [projp.tile([128, 512], bf16, name=f"yTL{i}")
                   for i in range(4)]
            state = {"read": False}

            def read_early(j):
                # after the second slot-1 chunk the early AllGather has
                # landed; pull my 4 early sender-chunks (opposite half).
                if state["read"] or j > 2:
                    return
                state["read"] = True
                for i in range(4):
                    nc.gpsimd.dma_start(yTE[i][:], agE_out[opp * 4 + i, myj])

            # slot-1 attention (exp-bound; no dummy filler - HAM throttle)
            emit_attn(1, iter(()), 0, post_chunk=read_early)

            # late exchange, then late reads (own half)
            nc.gpsimd.collective_compute(
                "AllGather", mybir.AluOpType.bypass,
                replica_groups=[list(range(NCORES))],
                ins=[agL_in[:].opt()], outs=[agL_out[:].opt()],
            )
            for i in range(4):
                nc.gpsimd.dma_start(yTL[i][:], agL_out[half * 4 + i, myj])

            attn_psum_o.__exit__(None, None, None)
            attn_psum_s.__exit__(None, None, None)
            qkv_psum.__exit__(None, None, None)

            # ---- output projection on own 512-row shard --------------------
            # early part (4 opposite-half k-blocks) runs while the late
            # AllGathers drain; late part (4 own-half blocks) after.
            proj_psum = tc.tile_pool(name="psy", bufs=1, space="PSUM")
            psy = proj_psum.__enter__()
            pair_ps = {}
            for m in range(SHARD // 128):
                for n in range(2):
                    ps = psy.tile([128, 512], f32, name=f"ps_y{m}{n}",
                                  tag=f"ps_y{m}{n}")
                    pair_ps[(m, n)] = ps
                    for k in range(4):
                        nc.tensor.matmul(
                            ps[:],
                            yTE[k][:, 128 * m : 128 * (m + 1)],
                            wpE_sb[:, C * k + 512 * n : C * k + 512 * (n + 1)],
                            start=(k == 0), stop=False,
                        )
            for m in range(SHARD // 128):
                ysb = ysbp.tile([128, C], f32, name="ysb", tag="ysb")
                for n in range(2):
                    ps = pair_ps[(m, n)]
                    for k in range(4):
                        nc.tensor.matmul(
                            ps[:],
                            yTL[k][:, 128 * m : 128 * (m + 1)],
                            wpL_sb[:, C * k + 512 * n : C * k + 512 * (n + 1)],
                            start=False, stop=(k == 3),
                        )
                    nc.vector.tensor_tensor(
                        ysb[:, 512 * n : 512 * (n + 1)],
                        ps[:],
                        bpb[:, 512 * n : 512 * (n + 1)],
                        op=mybir.AluOpType.add,
                    )
                nc.sync.dma_start(y[128 * m : 128 * (m + 1), :], ysb[:])
            proj_psum.__exit__(None, None, None)

    nc.compile()
    return nc


def _get_nc():
    if "nc" not in _CACHE:
        _CACHE["nc"] = _build_nc()
    return _CACHE["nc"]


def kernel(x, W_attn, b_attn, W_proj, b_proj, _trace=False):
    global LAST_EXEC_NS
    from concourse.bass_utils import run_bass_kernel_spmd
    import ml_dtypes

    bf16 = ml_dtypes.bfloat16

    x = np.asarray(x, np.float32)
    W_attn = np.asarray(W_attn, np.float32)
    b_attn = np.asarray(b_attn, np.float32)
    W_proj = np.asarray(W_proj, np.float32)
    b_proj = np.asarray(b_proj, np.float32)

    def pmajor4(w):  # [512, M] -> [128, 4*M], block a at cols [a*M:(a+1)*M]
        m = w.shape[1]
        return np.ascontiguousarray(
            w.reshape(4, 128, m).transpose(1, 0, 2).reshape(128, 4 * m)
        ).astype(bf16)

    def pmajor(w):  # [C, M] -> [128, KT*M]
        m = w.shape[1]
        return np.ascontiguousarray(
            w.reshape(KT, 128, m).transpose(1, 0, 2).reshape(128, KT * m)
        ).astype(bf16)

    xT = np.transpose(x, (0, 2, 1))  # [B, C, T]
    xT16 = np.ascontiguousarray(
        xT.reshape(B, KT, 128, T).transpose(0, 2, 1, 3).reshape(B, 128, KT * T)
    ).astype(bf16)
    bp_h = np.ascontiguousarray(b_proj).reshape(1, C)

    # W_proj k-blocks: block k = rows [128k, 128k+128) (channels of core k)
    wp_blocks = W_proj.reshape(KT, 128, C)

    in_maps = []
    for c in range(NCORES):
        half = c // 4
        s = slice(CW * c, CW * (c + 1))
        # early = opposite-half senders' blocks, late = own-half senders'
        wpE_h = pmajor4(
            wp_blocks[4 * (1 - half) : 4 * (1 - half) + 4].reshape(512, C))
        wpL_h = pmajor4(wp_blocks[4 * half : 4 * half + 4].reshape(512, C))
        # slot 0 = opposite half's batch, slot 1 = own batch
        x_slots = np.stack([xT16[1 - half], xT16[half]])
        in_maps.append({
            "xT": x_slots,
            "wq": pmajor(W_attn[:, s]),
            "wk": pmajor(W_attn[:, C:][:, s]),
            "wv": pmajor(W_attn[:, 2 * C:][:, s]),
            "wpE": wpE_h,
            "wpL": wpL_h,
            "bq": np.ascontiguousarray(b_attn[s]).reshape(CW, 1),
            "bk": np.ascontiguousarray(b_attn[C:][s]).reshape(CW, 1),
            "bv": np.ascontiguousarray(b_attn[2 * C:][s]).reshape(CW, 1),
            "bp": bp_h,
        })

    nc = _get_nc()
    res = run_bass_kernel_spmd(nc, in_maps, list(range(NCORES)), trace=_trace)
    LAST_EXEC_NS = res.exec_time_ns

    out = np.empty((B, T, C), np.float32)
    for c in range(NCORES):
        out[c // 4, 512 * (c % 4) : 512 * (c % 4 + 1), :] = res.results[c]["y"]
    return out


# revision 8
# speedup vs baseline: 1.1898x; 1.1898x over previous
"""Multi-head causal attention (B=2, T=2048, C=1024, H=16) on 8 trn2 NeuronCores.

Sharding: 2 heads per core (tensor-parallel over heads), both batch elements
on every core. Per core: qkv projection for its 2 heads, flash-style causal
attention in the S^T = k q^T layout, then one AllToAll exchanging attention
outputs so each core owns 512 full-channel rows of (b, t), followed by the
output projection on that row shard.

Performance structure (v5):
  * bf16 operands (same PE rate as fp16, less multiplier toggling -> less
    HAM duty-cycle throttling), fp32 PSUM accumulation.
  * The scalar engine's exp cannot feed the PE at full clock, and PE idle
    gaps reset the p-state ramp, so slot-1's qkv projection is interleaved
    into slot-0's attention inner loop as useful PE filler.
  * Causal masking on GpSimd, divide-by-rowsum fused into the PSUM->SBUF
    eviction on Vector, so ScalarE runs pure exp.
  * x is DMA'd per k-tile; warm-up matmuls on local tiles ramp the PE
    p-state under the DMA; a tiny warm-up AllToAll pre-opens the collective
    channel so the main exchange starts without setup latency.
  * Collectives overlapped with compute run ~4x slower on this fabric and
    deepen the activity throttle, so the exchange is a single AllToAll after
    all attention compute.

Host side shards/transposes/casts inputs (bf16) and reassembles the output.
"""

import sys

import numpy as np

if "/opt/trn_rl_repo" not in sys.path:
    sys.path.insert(0, "/opt/trn_rl_repo")

B, T, C, H, D = 2, 2048, 1024, 16, 64
NCORES = 8
HPC = H // NCORES          # heads per core = 2
CW = HPC * D               # per-core channel width = 128
KT = C // 128              # k tiles = 8
TT = T // 128              # t tiles = 16
SHARD = (B * T) // NCORES  # output rows per core = 512
SCALE = 1.0 / float(np.sqrt(C))

_CACHE = {}
LAST_EXEC_NS = None


def _build_nc():
    import concourse.mybir as mybir
    import concourse.tile as tile
    from concourse import bacc
    from concourse.masks import make_identity, make_upper_triangular

    f32 = mybir.dt.float32
    bf16 = mybir.dt.bfloat16

    nc = bacc.Bacc("TRN2", target_bir_lowering=False, debug=False,
                   num_devices=NCORES)

    xT = nc.dram_tensor("xT", [2, 128, KT * T], bf16, kind="ExternalInput")
    wq = nc.dram_tensor("wq", [128, KT * CW], bf16, kind="ExternalInput")
    wk = nc.dram_tensor("wk", [128, KT * CW], bf16, kind="ExternalInput")
    wv = nc.dram_tensor("wv", [128, KT * CW], bf16, kind="ExternalInput")
    wp = nc.dram_tensor("wp", [128, KT * C], bf16, kind="ExternalInput")
    bq = nc.dram_tensor("bq", [CW, 1], f32, kind="ExternalInput")
    bk = nc.dram_tensor("bk", [CW, 1], f32, kind="ExternalInput")
    bv = nc.dram_tensor("bv", [CW, 1], f32, kind="ExternalInput")
    bp = nc.dram_tensor("bp", [1, C], f32, kind="ExternalInput")
    y = nc.dram_tensor("y", [SHARD, C], f32, kind="ExternalOutput")

    a2a_in = nc.dram_tensor("a2a_in", [NCORES, 128, 512], bf16)
    a2a_out = nc.dram_tensor("a2a_out", [NCORES, 128, 512], bf16)
    warm_i = nc.dram_tensor("warm_i", [8, 16], f32)
    warm_o = nc.dram_tensor("warm_o", [8, 16], f32)

    with tile.TileContext(nc) as tc:
        with (
            tc.tile_pool(name="const", bufs=1) as const,
            tc.tile_pool(name="xtp", bufs=1) as xtp,
            tc.tile_pool(name="wqkv", bufs=1) as wqkvp,
            tc.tile_pool(name="qkv", bufs=1) as qkvp,
            tc.tile_pool(name="pt", bufs=3) as ptp,
            tc.tile_pool(name="otp", bufs=1) as otp,
            tc.tile_pool(name="sm", bufs=1) as smp,
            tc.tile_pool(name="proj", bufs=1) as projp,
            tc.tile_pool(name="ysb", bufs=2) as ysbp,
        ):
            # ---- x DMA, split per k-tile so the first matmul starts early --
            xt = {}
            for s in range(2):
                xt[s] = [xtp.tile([128, T], bf16, name=f"xt{s}_{a}")
                         for a in range(KT)]
            for s in range(2):
                for a in range(KT):
                    nc.sync.dma_start(xt[s][a][:], xT[s, :, T * a : T * (a + 1)])

            # ---- collective warm-up (channel init overlaps compute) --------
            wtile = const.tile([8, 16], f32, name="wtile")
            nc.vector.memset(wtile[:], 0.0)
            nc.sync.dma_start(warm_i[:], wtile[:])
            nc.gpsimd.collective_compute(
                "AllToAll", mybir.AluOpType.bypass,
                replica_groups=[list(range(NCORES))],
                ins=[warm_i[:].opt()], outs=[warm_o[:].opt()],
            )

            # ---- constants -------------------------------------------------
            trimask = const.tile([128, 128], bf16, name="trimask")
            make_upper_triangular(nc, trimask[:], val=1.0, diag=True)
            ident = const.tile([128, 128], bf16, name="ident")
            make_identity(nc, ident[:])
            wscr = const.tile([128, 512], bf16, name="wscr")
            nc.vector.memset(wscr[:], 0.0)

            bq_t = const.tile([CW, 1], f32, name="bq_t")
            bk_t = const.tile([CW, 1], f32, name="bk_t")
            bv_t = const.tile([CW, 1], f32, name="bv_t")
            nc.sync.dma_start(bq_t[:], bq[:])
            nc.sync.dma_start(bk_t[:], bk[:])
            nc.sync.dma_start(bv_t[:], bv[:])
            bp_row = const.tile([1, C], f32, name="bp_row")
            nc.sync.dma_start(bp_row[:], bp[:])
            bpb = const.tile([128, C], f32, name="bpb")
            nc.gpsimd.partition_broadcast(bpb[:], bp_row[:])

            # ---- weights ---------------------------------------------------
            wq_sb = wqkvp.tile([128, KT * CW], bf16, name="wq_sb")
            wk_sb = wqkvp.tile([128, KT * CW], bf16, name="wk_sb")
            wv_sb = wqkvp.tile([128, KT * CW], bf16, name="wv_sb")
            nc.sync.dma_start(wq_sb[:], wq[:])
            nc.sync.dma_start(wk_sb[:], wk[:])
            nc.sync.dma_start(wv_sb[:], wv[:])
            wp_sb = projp.tile([128, KT * C], bf16, name="wp_sb")
            nc.sync.dma_start(wp_sb[:], wp[:])

            # ---- psum pools ------------------------------------------------
            qkv_psum = tc.tile_pool(name="psqk", bufs=2, space="PSUM")
            psqk = qkv_psum.__enter__()
            attn_psum_s = tc.tile_pool(name="ps_s", bufs=2, space="PSUM")
            ps_s = attn_psum_s.__enter__()
            attn_psum_o = tc.tile_pool(name="ps_o", bufs=1, space="PSUM")
            ps_o = attn_psum_o.__enter__()

            # ---- PE p-state ramp: warm-ups on local tiles (no DMA gate) ----
            warm_ps = psqk.tile([128, 512], f32, name="warm_ps", tag="ps_qk")
            for _ in range(36):
                nc.tensor.matmul(
                    warm_ps[:], trimask[:], wscr[:], start=True, stop=True,
                )
            nc.vector.memset(warm_ps[:, 0:2], 0.0)

            qT_sb, kT_sb, v_sb, ot_sb, r_all = {}, {}, {}, {}, {}

            def qkv_steps(s):
                """Generator: emits qkv projection for slot s, yielding after
                each PE instruction so it can interleave with attention."""
                qT_b = qkvp.tile([128, T], bf16, name=f"qT{s}")
                kT_b = qkvp.tile([128, T], bf16, name=f"kT{s}")
                qT_sb[s], kT_sb[s] = qT_b, kT_b
                for dst, w_sb, bias in ((qT_b, wq_sb, bq_t), (kT_b, wk_sb, bk_t)):
                    for j in range(4):
                        ps = psqk.tile([128, 512], f32, name="ps_qk", tag="ps_qk")
                        for a in range(KT):
                            nc.tensor.matmul(
                                ps[:],
                                w_sb[:, CW * a : CW * (a + 1)],
                                xt[s][a][:, 512 * j : 512 * (j + 1)],
                                start=(a == 0), stop=(a == KT - 1),
                            )
                            yield
                        nc.vector.tensor_scalar_add(
                            dst[:, 512 * j : 512 * (j + 1)], ps[:], bias[:]
                        )
                vT_b = qkvp.tile([128, T], bf16, name=f"vT{s}")
                for j in range(4):
                    ps = psqk.tile([128, 512], f32, name="ps_vT", tag="ps_qk")
                    for a in range(KT):
                        nc.tensor.matmul(
                            ps[:],
                            wv_sb[:, CW * a : CW * (a + 1)],
                            xt[s][a][:, 512 * j : 512 * (j + 1)],
                            start=(a == 0), stop=(a == KT - 1),
                        )
                        yield
                    nc.vector.tensor_scalar_add(
                        vT_b[:, 512 * j : 512 * (j + 1)], ps[:], bv_t[:]
                    )
                v_b = []
                for m in range(TT):
                    vt = qkvp.tile([128, 2 * (D + 1)], bf16, name=f"v{s}_{m}")
                    tps = psqk.tile([128, 128], bf16, name="ps_tr", tag="ps_qk")
                    nc.tensor.transpose(
                        tps[:], vT_b[:, 128 * m : 128 * (m + 1)], ident[:]
                    )
                    yield
                    nc.vector.tensor_copy(
                        vt[:].rearrange("p (a m) -> p a m", a=2)[:, :, 0:D],
                        tps[:].rearrange("p (a m) -> p a m", a=2),
                    )
                    nc.vector.memset(vt[:, D : D + 1], 1.0)
                    nc.vector.memset(vt[:, 2 * D + 1 : 2 * D + 2], 1.0)
                    v_b.append(vt)
                v_sb[s] = v_b

            def filler_dummy():
                """Endless PE keep-busy matmuls into a cycled psum tile."""
                while True:
                    ps = psqk.tile([128, 512], f32, name="ps_dmy", tag="ps_qk")
                    nc.tensor.matmul(
                        ps[:], trimask[:], wscr[:], start=True, stop=True,
                    )
                    yield

            def emit_attn(s, filler, fill_per_step):
                ot = otp.tile([128, T], bf16, name=f"ot{s}")
                ot_sb[s] = ot
                ra = smp.tile([1, 4096], f32, name=f"r_all{s}", tag="r_all")
                r_all[s] = ra
                for j in (3, 2, 1, 0):
                    o_ps = [
                        ps_o.tile([65, 512], f32, name=f"o{h}", tag=f"o{h}")
                        for h in range(2)
                    ]
                    ilast = 4 * (j + 1) - 1
                    for i in range(4 * (j + 1)):
                        off = max(0, 128 * i - 512 * j)
                        s_ps = ps_s.tile([128, 1024], f32, name="s_ps", tag="s")
                        pt = ptp.tile([128, 1024], bf16, name="pt", tag="pt")
                        for h in range(2):
                            nc.tensor.matmul(
                                s_ps[:, 512 * h + off : 512 * (h + 1)],
                                kT_sb[s][64 * h : 64 * h + 64,
                                         128 * i : 128 * (i + 1)],
                                qT_sb[s][64 * h : 64 * h + 64,
                                         512 * j + off : 512 * (j + 1)],
                                start=True, stop=True,
                            )
                        # PE filler while ScalarE computes the exp
                        for _ in range(fill_per_step):
                            next(filler, None)
                        nc.scalar.activation(
                            pt[:].rearrange("p (g w) -> p g w", g=2)[:, :, off:512],
                            s_ps[:].rearrange("p (g w) -> p g w", g=2)[:, :, off:512],
                            mybir.ActivationFunctionType.Exp,
                            scale=SCALE,
                        )
                        if 4 * j <= i:
                            for h in range(2):
                                nc.gpsimd.tensor_tensor(
                                    pt[:, 512 * h + off : 512 * h + off + 128],
                                    pt[:, 512 * h + off : 512 * h + off + 128],
                                    trimask[:],
                                    op=mybir.AluOpType.mult,
                                )
                        for h in range(2):
                            nc.tensor.matmul(
                                o_ps[h][0:65, off:512],
                                v_sb[s][i][:, (D + 1) * h : (D + 1) * (h + 1)],
                                pt[:, 512 * h + off : 512 * (h + 1)],
                                start=(i == 0), stop=(i == ilast),
                            )
                    # rowsums, reciprocal, fused divide+evict for chunk j
                    for h in range(2):
                        idx = 2 * j + h
                        nc.vector.tensor_copy(
                            ra[0:1, 512 * idx : 512 * (idx + 1)],
                            o_ps[h][64:65, :],
                        )
                    rs = ra[0:1, 1024 * j : 1024 * j + 1024]
                    nc.vector.reciprocal_approx_fast(rs, rs)
                    rb = smp.tile([128, 1024], f32, name="rb", tag="rb", bufs=2)
                    nc.gpsimd.partition_broadcast(rb[:], rs)
                    for h in range(2):
                        nc.vector.tensor_tensor(
                            ot[64 * h : 64 * h + 64, 512 * j : 512 * (j + 1)],
                            o_ps[h][0:64, :],
                            rb[64 * h : 64 * h + 64, 512 * h : 512 * (h + 1)],
                            op=mybir.AluOpType.mult,
                        )
                    nc.sync.dma_start(a2a_in[4 * s + j],
                                       ot[:, 512 * j : 512 * (j + 1)])

            # slot-0 qkv runs standalone (PE-dense)
            for _ in qkv_steps(0):
                pass
            # slot-0 attention interleaves slot-1 qkv as PE filler
            g1 = qkv_steps(1)
            emit_attn(0, g1, 3)
            for _ in g1:  # finish any qkv(1) remainder
                pass

            # slot-1 attention (exp-bound; no dummy filler - HAM throttle)
            emit_attn(1, iter(()), 0)

            # ---- exchange: AllToAll over all 8 cores ----
            nc.gpsimd.collective_compute(
                "AllToAll", mybir.AluOpType.bypass,
                replica_groups=[list(range(NCORES))],
                ins=[a2a_in[:].opt()], outs=[a2a_out[:].opt()],
            )

            attn_psum_o.__exit__(None, None, None)
            attn_psum_s.__exit__(None, None, None)
            qkv_psum.__exit__(None, None, None)

            # ---- output projection on own 512-row shard --------------------
            proj_psum = tc.tile_pool(name="psy", bufs=3, space="PSUM")
            psy = proj_psum.__enter__()
            yT_sb = []
            for k in range(KT):
                yt = projp.tile([128, 512], bf16, name=f"yT{k}")
                nc.sync.dma_start(yt[:], a2a_out[k])
                yT_sb.append(yt)
            for m in range(SHARD // 128):
                ysb = ysbp.tile([128, C], f32, name="ysb", tag="ysb")
                for n in range(2):
                    ps = psy.tile([128, 512], f32, name="ps_y", tag="ps_y")
                    for k in range(KT):
                        nc.tensor.matmul(
                            ps[:],
                            yT_sb[k][:, 128 * m : 128 * (m + 1)],
                            wp_sb[:, C * k + 512 * n : C * k + 512 * (n + 1)],
                            start=(k == 0), stop=(k == KT - 1),
                        )
                    nc.vector.tensor_tensor(
                        ysb[:, 512 * n : 512 * (n + 1)],
                        ps[:],
                        bpb[:, 512 * n : 512 * (n + 1)],
                        op=mybir.AluOpType.add,
                    )
                nc.sync.dma_start(y[128 * m : 128 * (m + 1), :], ysb[:])
            proj_psum.__exit__(None, None, None)

    nc.compile()
    return nc


def _get_nc():
    if "nc" not in _CACHE:
        _CACHE["nc"] = _build_nc()
    return _CACHE["nc"]


def kernel(x, W_attn, b_attn, W_proj, b_proj, _trace=False):
    global LAST_EXEC_NS
    from concourse.bass_utils import run_bass_kernel_spmd
    import ml_dtypes

    bf16 = ml_dtypes.bfloat16

    x = np.asarray(x, np.float32)
    W_attn = np.asarray(W_attn, np.float32)
    b_attn = np.asarray(b_attn, np.float32)
    W_proj = np.asarray(W_proj, np.float32)
    b_proj = np.asarray(b_proj, np.float32)

    def pmajor(w):  # [C, M] -> [128, KT*M]
        m = w.shape[1]
        return np.ascontiguousarray(
            w.reshape(KT, 128, m).transpose(1, 0, 2).reshape(128, KT * m)
        ).astype(bf16)

    xT = np.transpose(x, (0, 2, 1))  # [B, C, T]
    xT16 = np.ascontiguousarray(
        xT.reshape(B, KT, 128, T).transpose(0, 2, 1, 3).reshape(B, 128, KT * T)
    ).astype(bf16)
    bp_h = np.ascontiguousarray(b_proj).reshape(1, C)

    wp16 = pmajor(W_proj)

    in_maps = []
    for c in range(NCORES):
        s_ = slice(CW * c, CW * (c + 1))
        in_maps.append({
            "xT": xT16,
            "wq": pmajor(W_attn[:, s_]),
            "wk": pmajor(W_attn[:, C:][:, s_]),
            "wv": pmajor(W_attn[:, 2 * C:][:, s_]),
            "wp": wp16,
            "bq": np.ascontiguousarray(b_attn[s_]).reshape(CW, 1),
            "bk": np.ascontiguousarray(b_attn[C:][s_]).reshape(CW, 1),
            "bv": np.ascontiguousarray(b_attn[2 * C:][s_]).reshape(CW, 1),
            "bp": bp_h,
        })

    nc = _get_nc()
    res = run_bass_kernel_spmd(nc, in_maps, list(range(NCORES)), trace=_trace)
    LAST_EXEC_NS = res.exec_time_ns

    out = np.empty((B, T, C), np.float32)
    for c in range(NCORES):
        out[c // 4, 512 * (c % 4) : 512 * (c % 4 + 1), :] = res.results[c]["y"]
    return out


# revision 10
# speedup vs baseline: 1.5700x; 1.3196x over previous
"""Multi-head causal attention (B=2, T=2048, C=1024, H=16) on 8 trn2 NeuronCores.

Sharding: 2 heads per core (tensor-parallel over heads), both batch elements
on every core. Per core: qkv projection for its 2 heads, flash-style causal
attention in the S^T = k q^T layout, then one AllToAll exchanging attention
outputs so each core owns 512 full-channel rows of (b, t), followed by the
output projection on that row shard.

Performance structure (v5):
  * bf16 operands (same PE rate as fp16, less multiplier toggling -> less
    HAM duty-cycle throttling), fp32 PSUM accumulation.
  * The scalar engine's exp cannot feed the PE at full clock, and PE idle
    gaps reset the p-state ramp, so slot-1's qkv projection is interleaved
    into slot-0's attention inner loop as useful PE filler.
  * Causal masking on GpSimd, divide-by-rowsum fused into the PSUM->SBUF
    eviction on Vector, so ScalarE runs pure exp.
  * x is DMA'd per k-tile; warm-up matmuls on local tiles ramp the PE
    p-state under the DMA; a tiny warm-up AllToAll pre-opens the collective
    channel so the main exchange starts without setup latency.
  * Collectives overlapped with compute run ~4x slower on this fabric and
    deepen the activity throttle, so the exchange is a single AllToAll after
    all attention compute.

Host side shards/transposes/casts inputs (bf16) and reassembles the output.
"""

import sys

import numpy as np

if "/opt/trn_rl_repo" not in sys.path:
    sys.path.insert(0, "/opt/trn_rl_repo")

B, T, C, H, D = 2, 2048, 1024, 16, 64
NCORES = 8
HPC = H // NCORES          # heads per core = 2
CW = HPC * D               # per-core channel width = 128
KT = C // 128              # k tiles = 8
TT = T // 128              # t tiles = 16
SHARD = (B * T) // NCORES  # output rows per core = 512
SCALE = 1.0 / float(np.sqrt(C))

_CACHE = {}
LAST_EXEC_NS = None


def _build_nc():
    import concourse.mybir as mybir
    import concourse.tile as tile
    from concourse import bacc
    from concourse.masks import make_identity, make_upper_triangular

    f32 = mybir.dt.float32
    bf16 = mybir.dt.float16  # fp16: lighter HAM throttle than bf16 observed

    nc = bacc.Bacc("TRN2", target_bir_lowering=False, debug=False,
                   num_devices=NCORES)

    xT = nc.dram_tensor("xT", [2, 128, KT * T], bf16, kind="ExternalInput")
    wq = nc.dram_tensor("wq", [128, KT * CW], bf16, kind="ExternalInput")
    wk = nc.dram_tensor("wk", [128, KT * CW], bf16, kind="ExternalInput")
    wv = nc.dram_tensor("wv", [128, KT * CW], bf16, kind="ExternalInput")
    wp = nc.dram_tensor("wp", [128, KT * C], bf16, kind="ExternalInput")
    bq = nc.dram_tensor("bq", [CW, 1], f32, kind="ExternalInput")
    bk = nc.dram_tensor("bk", [CW, 1], f32, kind="ExternalInput")
    bv = nc.dram_tensor("bv", [CW, 1], f32, kind="ExternalInput")
    bp = nc.dram_tensor("bp", [1, C], f32, kind="ExternalInput")
    y = nc.dram_tensor("y", [SHARD, C], f32, kind="ExternalOutput")

    a2a_in = nc.dram_tensor("a2a_in", [NCORES, 128, 512], bf16)
    a2a_out = nc.dram_tensor("a2a_out", [NCORES, 128, 512], bf16)
    warm_i = nc.dram_tensor("warm_i", [8, 16], f32)
    warm_o = nc.dram_tensor("warm_o", [8, 16], f32)

    with tile.TileContext(nc) as tc:
        with (
            tc.tile_pool(name="const", bufs=1) as const,
            tc.tile_pool(name="xtp", bufs=1) as xtp,
            tc.tile_pool(name="wqkv", bufs=1) as wqkvp,
            tc.tile_pool(name="qkv", bufs=1) as qkvp,
            tc.tile_pool(name="pt", bufs=3) as ptp,
            tc.tile_pool(name="otp", bufs=1) as otp,
            tc.tile_pool(name="sm", bufs=1) as smp,
            tc.tile_pool(name="proj", bufs=1) as projp,
            tc.tile_pool(name="ysb", bufs=2) as ysbp,
        ):
            # ---- x DMA, split per k-tile so the first matmul starts early --
            xt = {}
            for s in range(2):
                xt[s] = [xtp.tile([128, T], bf16, name=f"xt{s}_{a}")
                         for a in range(KT)]
            for s in range(2):
                for a in range(KT):
                    nc.sync.dma_start(xt[s][a][:], xT[s, :, T * a : T * (a + 1)])

            # ---- collective warm-up (channel init overlaps compute) --------
            wtile = const.tile([8, 16], f32, name="wtile")
            nc.vector.memset(wtile[:], 0.0)
            nc.sync.dma_start(warm_i[:], wtile[:])
            nc.gpsimd.collective_compute(
                "AllToAll", mybir.AluOpType.bypass,
                replica_groups=[list(range(NCORES))],
                ins=[warm_i[:].opt()], outs=[warm_o[:].opt()],
            )

            # ---- constants -------------------------------------------------
            trimask = const.tile([128, 128], bf16, name="trimask")
            make_upper_triangular(nc, trimask[:], val=1.0, diag=True)
            ident = const.tile([128, 128], bf16, name="ident")
            make_identity(nc, ident[:])
            wscr = const.tile([128, 512], bf16, name="wscr")
            nc.vector.memset(wscr[:], 0.0)

            bq_t = const.tile([CW, 1], f32, name="bq_t")
            bk_t = const.tile([CW, 1], f32, name="bk_t")
            bv_t = const.tile([CW, 1], f32, name="bv_t")
            nc.sync.dma_start(bq_t[:], bq[:])
            nc.sync.dma_start(bk_t[:], bk[:])
            nc.sync.dma_start(bv_t[:], bv[:])
            bp_row = const.tile([1, C], f32, name="bp_row")
            nc.sync.dma_start(bp_row[:], bp[:])
            bpb = const.tile([128, C], f32, name="bpb")
            nc.gpsimd.partition_broadcast(bpb[:], bp_row[:])

            # ---- weights ---------------------------------------------------
            wq_sb = wqkvp.tile([128, KT * CW], bf16, name="wq_sb")
            wk_sb = wqkvp.tile([128, KT * CW], bf16, name="wk_sb")
            wv_sb = wqkvp.tile([128, KT * CW], bf16, name="wv_sb")
            nc.sync.dma_start(wq_sb[:], wq[:])
            nc.sync.dma_start(wk_sb[:], wk[:])
            nc.sync.dma_start(wv_sb[:], wv[:])
            wp_sb = projp.tile([128, KT * C], bf16, name="wp_sb")
            nc.sync.dma_start(wp_sb[:], wp[:])

            # ---- psum pools ------------------------------------------------
            attn_psum_s = tc.tile_pool(name="ps_s", bufs=2, space="PSUM")
            ps_s = attn_psum_s.__enter__()
            qkv_psum = tc.tile_pool(name="psqk", bufs=2, space="PSUM")
            psqk = qkv_psum.__enter__()
            attn_psum_o = tc.tile_pool(name="ps_o", bufs=1, space="PSUM")
            ps_o = attn_psum_o.__enter__()

            # ---- PE p-state ramp: warm-ups on local tiles (no DMA gate) ----
            warm_ps = psqk.tile([128, 512], f32, name="warm_ps", tag="ps_qk")
            for _ in range(44):
                nc.tensor.matmul(
                    warm_ps[:], trimask[:], wscr[:], start=True, stop=True,
                )
            nc.vector.memset(warm_ps[:, 0:2], 0.0)

            qT_sb, kT_sb, v_sb, ot_sb, r_all = {}, {}, {}, {}, {}

            def qkv_steps(s):
                """Generator: emits qkv projection for slot s, yielding after
                each PE instruction so it can interleave with attention."""
                qT_b = qkvp.tile([128, T], bf16, name=f"qT{s}")
                kT_b = qkvp.tile([128, T], bf16, name=f"kT{s}")
                qT_sb[s], kT_sb[s] = qT_b, kT_b
                for dst, w_sb, bias in ((qT_b, wq_sb, bq_t), (kT_b, wk_sb, bk_t)):
                    for j in range(4):
                        ps = psqk.tile([128, 512], f32, name="ps_qk", tag="ps_qk")
                        for a in range(KT):
                            nc.tensor.matmul(
                                ps[:],
                                w_sb[:, CW * a : CW * (a + 1)],
                                xt[s][a][:, 512 * j : 512 * (j + 1)],
                                start=(a == 0), stop=(a == KT - 1),
                            )
                            yield
                        nc.vector.tensor_scalar_add(
                            dst[:, 512 * j : 512 * (j + 1)], ps[:], bias[:]
                        )
                vT_b = qkvp.tile([128, T], bf16, name=f"vT{s}")
                for j in range(4):
                    ps = psqk.tile([128, 512], f32, name="ps_vT", tag="ps_qk")
                    for a in range(KT):
                        nc.tensor.matmul(
                            ps[:],
                            wv_sb[:, CW * a : CW * (a + 1)],
                            xt[s][a][:, 512 * j : 512 * (j + 1)],
                            start=(a == 0), stop=(a == KT - 1),
                        )
                        yield
                    nc.vector.tensor_scalar_add(
                        vT_b[:, 512 * j : 512 * (j + 1)], ps[:], bv_t[:]
                    )
                v_b = []
                for m in range(TT):
                    vt = qkvp.tile([128, 2 * (D + 1)], bf16, name=f"v{s}_{m}")
                    tps = psqk.tile([128, 128], bf16, name="ps_tr", tag="ps_qk")
                    nc.tensor.transpose(
                        tps[:], vT_b[:, 128 * m : 128 * (m + 1)], ident[:]
                    )
                    yield
                    nc.vector.tensor_copy(
                        vt[:].rearrange("p (a m) -> p a m", a=2)[:, :, 0:D],
                        tps[:].rearrange("p (a m) -> p a m", a=2),
                    )
                    nc.vector.memset(vt[:, D : D + 1], 1.0)
                    nc.vector.memset(vt[:, 2 * D + 1 : 2 * D + 2], 1.0)
                    v_b.append(vt)
                v_sb[s] = v_b

            def filler_dummy():
                """Endless PE keep-busy matmuls into a cycled psum tile."""
                while True:
                    ps = psqk.tile([128, 512], f32, name="ps_dmy", tag="ps_qk")
                    nc.tensor.matmul(
                        ps[:], trimask[:], wscr[:], start=True, stop=True,
                    )
                    yield

            def emit_attn(s, filler, fill_per_step, o_pool, o_bufs):
                ot = otp.tile([128, T], bf16, name=f"ot{s}")
                ot_sb[s] = ot
                ra = smp.tile([1, 4096], f32, name=f"r_all{s}", tag="r_all")
                r_all[s] = ra
                for j in (3, 2, 1, 0):
                    o_ps = [
                        o_pool.tile([65, 512], f32, name=f"o{h}", tag=f"o{h}",
                                    bufs=o_bufs)
                        for h in range(2)
                    ]
                    ilast = 4 * (j + 1) - 1
                    for i in range(4 * (j + 1)):
                        off = max(0, 128 * i - 512 * j)
                        s_ps = ps_s.tile([128, 1024], f32, name="s_ps", tag="s")
                        pt = ptp.tile([128, 1024], bf16, name="pt", tag="pt")
                        for h in range(2):
                            nc.tensor.matmul(
                                s_ps[:, 512 * h + off : 512 * (h + 1)],
                                kT_sb[s][64 * h : 64 * h + 64,
                                         128 * i : 128 * (i + 1)],
                                qT_sb[s][64 * h : 64 * h + 64,
                                         512 * j + off : 512 * (j + 1)],
                                start=True, stop=True,
                            )
                        # PE filler while ScalarE computes the exp
                        for _ in range(fill_per_step):
                            next(filler, None)
                        nc.scalar.activation(
                            pt[:].rearrange("p (g w) -> p g w", g=2)[:, :, off:512],
                            s_ps[:].rearrange("p (g w) -> p g w", g=2)[:, :, off:512],
                            mybir.ActivationFunctionType.Exp,
                            scale=SCALE,
                        )
                        if 4 * j <= i:
                            for h in range(2):
                                nc.vector.tensor_tensor(
                                    pt[:, 512 * h + off : 512 * h + off + 128],
                                    pt[:, 512 * h + off : 512 * h + off + 128],
                                    trimask[:],
                                    op=mybir.AluOpType.mult,
                                )
                        for h in range(2):
                            nc.tensor.matmul(
                                o_ps[h][0:65, off:512],
                                v_sb[s][i][:, (D + 1) * h : (D + 1) * (h + 1)],
                                pt[:, 512 * h + off : 512 * (h + 1)],
                                start=(i == 0), stop=(i == ilast),
                            )
                    # rowsums, reciprocal, fused divide+evict for chunk j
                    for h in range(2):
                        idx = 2 * j + h
                        nc.vector.tensor_copy(
                            ra[0:1, 512 * idx : 512 * (idx + 1)],
                            o_ps[h][64:65, :],
                        )
                    rs = ra[0:1, 1024 * j : 1024 * j + 1024]
                    nc.vector.reciprocal_approx_fast(rs, rs)
                    rb = smp.tile([128, 1024], f32, name="rb", tag="rb", bufs=2)
                    nc.gpsimd.partition_broadcast(rb[:], rs)
                    for h in range(2):
                        nc.vector.tensor_tensor(
                            ot[64 * h : 64 * h + 64, 512 * j : 512 * (j + 1)],
                            o_ps[h][0:64, :],
                            rb[64 * h : 64 * h + 64, 512 * h : 512 * (h + 1)],
                            op=mybir.AluOpType.mult,
                        )
                    nc.sync.dma_start(a2a_in[4 * s + j],
                                       ot[:, 512 * j : 512 * (j + 1)])

            # slot-0 qkv runs standalone (PE-dense)
            for _ in qkv_steps(0):
                pass
            # slot-0 attention interleaves slot-1 qkv as PE filler
            g1 = qkv_steps(1)
            emit_attn(0, g1, 3, ps_o, 1)
            for _ in g1:  # finish any qkv(1) remainder
                pass

            # slot-1: free the qkv psum banks, double-buffer o_ps so the
            # divide chain at chunk boundaries overlaps the next chunk's PV
            attn_psum_o.__exit__(None, None, None)
            qkv_psum.__exit__(None, None, None)
            attn_psum_o2 = tc.tile_pool(name="ps_o2", bufs=2, space="PSUM")
            ps_o2 = attn_psum_o2.__enter__()
            emit_attn(1, iter(()), 0, ps_o2, 2)

            # ---- exchange: AllToAll over all 8 cores ----
            nc.gpsimd.collective_compute(
                "AllToAll", mybir.AluOpType.bypass,
                replica_groups=[list(range(NCORES))],
                ins=[a2a_in[:].opt()], outs=[a2a_out[:].opt()],
            )

            attn_psum_o2.__exit__(None, None, None)
            attn_psum_s.__exit__(None, None, None)

            # ---- output projection on own 512-row shard --------------------
            proj_psum = tc.tile_pool(name="psy", bufs=3, space="PSUM")
            psy = proj_psum.__enter__()
            yT_sb = []
            for k in range(KT):
                yt = projp.tile([128, 512], bf16, name=f"yT{k}")
                nc.sync.dma_start(yt[:], a2a_out[k])
                yT_sb.append(yt)
            for m in range(SHARD // 128):
                ysb = ysbp.tile([128, C], f32, name="ysb", tag="ysb")
                for n in range(2):
                    ps = psy.tile([128, 512], f32, name="ps_y", tag="ps_y")
                    for k in range(KT):
                        nc.tensor.matmul(
                            ps[:],
                            yT_sb[k][:, 128 * m : 128 * (m + 1)],
                            wp_sb[:, C * k + 512 * n : C * k + 512 * (n + 1)],
                            start=(k == 0), stop=(k == KT - 1),
                        )
                    nc.vector.tensor_tensor(
                        ysb[:, 512 * n : 512 * (n + 1)],
                        ps[:],
                        bpb[:, 512 * n : 512 * (n + 1)],
                        op=mybir.AluOpType.add,
                    )
                nc.sync.dma_start(y[128 * m : 128 * (m + 1), :], ysb[:])
            proj_psum.__exit__(None, None, None)

    nc.compile()
    return nc


def _get_nc():
    if "nc" not in _CACHE:
        _CACHE["nc"] = _build_nc()
    return _CACHE["nc"]


def kernel(x, W_attn, b_attn, W_proj, b_proj, _trace=False):
    global LAST_EXEC_NS
    from concourse.bass_utils import run_bass_kernel_spmd
    bf16 = np.float16

    x = np.asarray(x, np.float32)
    W_attn = np.asarray(W_attn, np.float32)
    b_attn = np.asarray(b_attn, np.float32)
    W_proj = np.asarray(W_proj, np.float32)
    b_proj = np.asarray(b_proj, np.float32)

    def pmajor(w):  # [C, M] -> [128, KT*M]
        m = w.shape[1]
        return np.ascontiguousarray(
            w.reshape(KT, 128, m).transpose(1, 0, 2).reshape(128, KT * m)
        ).astype(bf16)

    xT = np.transpose(x, (0, 2, 1))  # [B, C, T]
    xT16 = np.ascontiguousarray(
        xT.reshape(B, KT, 128, T).transpose(0, 2, 1, 3).reshape(B, 128, KT * T)
    ).astype(bf16)
    bp_h = np.ascontiguousarray(b_proj).reshape(1, C)

    wp16 = pmajor(W_proj)

    in_maps = []
    for c in range(NCORES):
        s_ = slice(CW * c, CW * (c + 1))
        in_maps.append({
            "xT": xT16,
            "wq": pmajor(W_attn[:, s_]),
            "wk": pmajor(W_attn[:, C:][:, s_]),
            "wv": pmajor(W_attn[:, 2 * C:][:, s_]),
            "wp": wp16,
            "bq": np.ascontiguousarray(b_attn[s_]).reshape(CW, 1),
            "bk": np.ascontiguousarray(b_attn[C:][s_]).reshape(CW, 1),
            "bv": np.ascontiguousarray(b_attn[2 * C:][s_]).reshape(CW, 1),
            "bp": bp_h,
        })

    nc = _get_nc()
    res = run_bass_kernel_spmd(nc, in_maps, list(range(NCORES)), trace=_trace)
    LAST_EXEC_NS = res.exec_time_ns

    out = np.empty((B, T, C), np.float32)
    for c in range(NCORES):
        out[c // 4, 512 * (c % 4) : 512 * (c % 4 + 1), :] = res.results[c]["y"]
    return out


# revision 11
# speedup vs baseline: 1.6500x; 1.0510x over previous
"""Multi-head causal attention (B=2, T=2048, C=1024, H=16) on 8 trn2 NeuronCores.

Sharding: 2 heads per core (tensor-parallel over heads), both batch elements
on every core. Per core: qkv projection for its 2 heads, flash-style causal
attention in the S^T = k q^T layout, then one AllToAll exchanging attention
outputs so each core owns 512 full-channel rows of (b, t), followed by the
output projection on that row shard.

Performance structure (v5):
  * bf16 operands (same PE rate as fp16, less multiplier toggling -> less
    HAM duty-cycle throttling), fp32 PSUM accumulation.
  * The scalar engine's exp cannot feed the PE at full clock, and PE idle
    gaps reset the p-state ramp, so slot-1's qkv projection is interleaved
    into slot-0's attention inner loop as useful PE filler.
  * Causal masking on GpSimd, divide-by-rowsum fused into the PSUM->SBUF
    eviction on Vector, so ScalarE runs pure exp.
  * x is DMA'd per k-tile; warm-up matmuls on local tiles ramp the PE
    p-state under the DMA; a tiny warm-up AllToAll pre-opens the collective
    channel so the main exchange starts without setup latency.
  * Collectives overlapped with compute run ~4x slower on this fabric and
    deepen the activity throttle, so the exchange is a single AllToAll after
    all attention compute.

Host side shards/transposes/casts inputs (bf16) and reassembles the output.
"""

import sys

import numpy as np

if "/opt/trn_rl_repo" not in sys.path:
    sys.path.insert(0, "/opt/trn_rl_repo")

B, T, C, H, D = 2, 2048, 1024, 16, 64
NCORES = 8
HPC = H // NCORES          # heads per core = 2
CW = HPC * D               # per-core channel width = 128
KT = C // 128              # k tiles = 8
TT = T // 128              # t tiles = 16
SHARD = (B * T) // NCORES  # output rows per core = 512
SCALE = 1.0 / float(np.sqrt(C))

_CACHE = {}
LAST_EXEC_NS = None


def _build_nc():
    import concourse.mybir as mybir
    import concourse.tile as tile
    from concourse import bacc
    from concourse.masks import make_identity, make_upper_triangular

    f32 = mybir.dt.float32
    bf16 = mybir.dt.float16  # fp16: lighter HAM throttle than bf16 observed

    nc = bacc.Bacc("TRN2", target_bir_lowering=False, debug=False,
                   num_devices=NCORES)

    xT = nc.dram_tensor("xT", [2, 128, KT * T], bf16, kind="ExternalInput")
    wq = nc.dram_tensor("wq", [128, KT * CW], bf16, kind="ExternalInput")
    wk = nc.dram_tensor("wk", [128, KT * CW], bf16, kind="ExternalInput")
    wv = nc.dram_tensor("wv", [128, KT * CW], bf16, kind="ExternalInput")
    wp = nc.dram_tensor("wp", [128, KT * C], bf16, kind="ExternalInput")
    bq = nc.dram_tensor("bq", [CW, 1], f32, kind="ExternalInput")
    bk = nc.dram_tensor("bk", [CW, 1], f32, kind="ExternalInput")
    bv = nc.dram_tensor("bv", [CW, 1], f32, kind="ExternalInput")
    bp = nc.dram_tensor("bp", [1, C], f32, kind="ExternalInput")
    y = nc.dram_tensor("y", [SHARD, C], f32, kind="ExternalOutput")

    a2a_in = nc.dram_tensor("a2a_in", [NCORES, 128, 512], bf16)
    a2a_out = nc.dram_tensor("a2a_out", [NCORES, 128, 512], bf16)
    warm_i = nc.dram_tensor("warm_i", [8, 16], f32)
    warm_o = nc.dram_tensor("warm_o", [8, 16], f32)

    with tile.TileContext(nc) as tc:
        with (
            tc.tile_pool(name="const", bufs=1) as const,
            tc.tile_pool(name="xtp", bufs=1) as xtp,
            tc.tile_pool(name="wqkv", bufs=1) as wqkvp,
            tc.tile_pool(name="qkv", bufs=1) as qkvp,
            tc.tile_pool(name="pt", bufs=3) as ptp,
            tc.tile_pool(name="otp", bufs=1) as otp,
            tc.tile_pool(name="sm", bufs=1) as smp,
            tc.tile_pool(name="proj", bufs=1) as projp,
            tc.tile_pool(name="ysb", bufs=2) as ysbp,
        ):
            # ---- x DMA, split per k-tile so the first matmul starts early --
            xt = {}
            for s in range(2):
                xt[s] = [xtp.tile([128, T], bf16, name=f"xt{s}_{a}")
                         for a in range(KT)]
            for s in range(2):
                for a in range(KT):
                    nc.sync.dma_start(xt[s][a][:], xT[s, :, T * a : T * (a + 1)])

            # ---- collective warm-up (channel init overlaps compute) --------
            wtile = const.tile([8, 16], f32, name="wtile")
            nc.vector.memset(wtile[:], 0.0)
            nc.sync.dma_start(warm_i[:], wtile[:])
            nc.gpsimd.collective_compute(
                "AllToAll", mybir.AluOpType.bypass,
                replica_groups=[list(range(NCORES))],
                ins=[warm_i[:].opt()], outs=[warm_o[:].opt()],
            )

            # ---- constants -------------------------------------------------
            trimask = const.tile([128, 128], bf16, name="trimask")
            make_upper_triangular(nc, trimask[:], val=1.0, diag=True)
            ident = const.tile([128, 128], bf16, name="ident")
            make_identity(nc, ident[:])
            wscr = const.tile([128, 512], bf16, name="wscr")
            nc.vector.memset(wscr[:], 0.0)

            bq_t = const.tile([CW, 1], f32, name="bq_t")
            bk_t = const.tile([CW, 1], f32, name="bk_t")
            bv_t = const.tile([CW, 1], f32, name="bv_t")
            nc.sync.dma_start(bq_t[:], bq[:])
            nc.sync.dma_start(bk_t[:], bk[:])
            nc.sync.dma_start(bv_t[:], bv[:])
            bp_row = const.tile([1, C], f32, name="bp_row")
            nc.sync.dma_start(bp_row[:], bp[:])
            bpb = const.tile([128, C], f32, name="bpb")
            nc.gpsimd.partition_broadcast(bpb[:], bp_row[:])

            # ---- weights ---------------------------------------------------
            wq_sb = wqkvp.tile([128, KT * CW], bf16, name="wq_sb")
            wk_sb = wqkvp.tile([128, KT * CW], bf16, name="wk_sb")
            wv_sb = wqkvp.tile([128, KT * CW], bf16, name="wv_sb")
            nc.sync.dma_start(wq_sb[:], wq[:])
            nc.sync.dma_start(wk_sb[:], wk[:])
            nc.sync.dma_start(wv_sb[:], wv[:])
            wp_sb = projp.tile([128, KT * C], bf16, name="wp_sb")
            nc.sync.dma_start(wp_sb[:], wp[:])

            # ---- psum pools ------------------------------------------------
            attn_psum_s = tc.tile_pool(name="ps_s", bufs=2, space="PSUM")
            ps_s = attn_psum_s.__enter__()
            qkv_psum = tc.tile_pool(name="psqk", bufs=2, space="PSUM")
            psqk = qkv_psum.__enter__()
            attn_psum_o = tc.tile_pool(name="ps_o", bufs=1, space="PSUM")
            ps_o = attn_psum_o.__enter__()

            # ---- PE p-state ramp: warm-ups on local tiles (no DMA gate) ----
            warm_ps = psqk.tile([128, 512], f32, name="warm_ps", tag="ps_qk")
            for _ in range(60):
                nc.tensor.matmul(
                    warm_ps[:], trimask[:], wscr[:], start=True, stop=True,
                )
            nc.vector.memset(warm_ps[:, 0:2], 0.0)

            qT_sb, kT_sb, v_sb, ot_sb, r_all = {}, {}, {}, {}, {}

            def qkv_steps(s):
                """Generator: emits qkv projection for slot s, yielding after
                each PE instruction so it can interleave with attention."""
                qT_b = qkvp.tile([128, T], bf16, name=f"qT{s}")
                kT_b = qkvp.tile([128, T], bf16, name=f"kT{s}")
                qT_sb[s], kT_sb[s] = qT_b, kT_b
                for dst, w_sb, bias in ((qT_b, wq_sb, bq_t), (kT_b, wk_sb, bk_t)):
                    for j in range(4):
                        ps = psqk.tile([128, 512], f32, name="ps_qk", tag="ps_qk")
                        for a in range(KT):
                            nc.tensor.matmul(
                                ps[:],
                                w_sb[:, CW * a : CW * (a + 1)],
                                xt[s][a][:, 512 * j : 512 * (j + 1)],
                                start=(a == 0), stop=(a == KT - 1),
                            )
                            yield
                        nc.vector.tensor_scalar_add(
                            dst[:, 512 * j : 512 * (j + 1)], ps[:], bias[:]
                        )
                vT_b = qkvp.tile([128, T], bf16, name=f"vT{s}")
                for j in range(4):
                    ps = psqk.tile([128, 512], f32, name="ps_vT", tag="ps_qk")
                    for a in range(KT):
                        nc.tensor.matmul(
                            ps[:],
                            wv_sb[:, CW * a : CW * (a + 1)],
                            xt[s][a][:, 512 * j : 512 * (j + 1)],
                            start=(a == 0), stop=(a == KT - 1),
                        )
                        yield
                    nc.vector.tensor_scalar_add(
                        vT_b[:, 512 * j : 512 * (j + 1)], ps[:], bv_t[:]
                    )
                v_b = []
                for m in range(TT):
                    vt = qkvp.tile([128, 2 * (D + 1)], bf16, name=f"v{s}_{m}")
                    tps = psqk.tile([128, 128], bf16, name="ps_tr", tag="ps_qk")
                    nc.tensor.transpose(
                        tps[:], vT_b[:, 128 * m : 128 * (m + 1)], ident[:]
                    )
                    yield
                    nc.vector.tensor_copy(
                        vt[:].rearrange("p (a m) -> p a m", a=2)[:, :, 0:D],
                        tps[:].rearrange("p (a m) -> p a m", a=2),
                    )
                    nc.vector.memset(vt[:, D : D + 1], 1.0)
                    nc.vector.memset(vt[:, 2 * D + 1 : 2 * D + 2], 1.0)
                    v_b.append(vt)
                v_sb[s] = v_b

            def filler_dummy():
                """Endless PE keep-busy matmuls into a cycled psum tile."""
                while True:
                    ps = psqk.tile([128, 512], f32, name="ps_dmy", tag="ps_qk")
                    nc.tensor.matmul(
                        ps[:], trimask[:], wscr[:], start=True, stop=True,
                    )
                    yield

            def emit_attn(s, filler, fill_per_step, o_pool, o_bufs):
                ot = otp.tile([128, T], bf16, name=f"ot{s}")
                ot_sb[s] = ot
                ra = smp.tile([1, 4096], f32, name=f"r_all{s}", tag="r_all")
                r_all[s] = ra
                for j in (3, 2, 1, 0):
                    o_ps = [
                        o_pool.tile([65, 512], f32, name=f"o{h}", tag=f"o{h}",
                                    bufs=o_bufs)
                        for h in range(2)
                    ]
                    ilast = 4 * (j + 1) - 1
                    for i in range(4 * (j + 1)):
                        off = max(0, 128 * i - 512 * j)
                        s_ps = ps_s.tile([128, 1024], f32, name="s_ps", tag="s")
                        pt = ptp.tile([128, 1024], bf16, name="pt", tag="pt")
                        for h in range(2):
                            nc.tensor.matmul(
                                s_ps[:, 512 * h + off : 512 * (h + 1)],
                                kT_sb[s][64 * h : 64 * h + 64,
                                         128 * i : 128 * (i + 1)],
                                qT_sb[s][64 * h : 64 * h + 64,
                                         512 * j + off : 512 * (j + 1)],
                                start=True, stop=True,
                            )
                        # PE filler while ScalarE computes the exp
                        for _ in range(fill_per_step):
                            next(filler, None)
                        nc.scalar.activation(
                            pt[:].rearrange("p (g w) -> p g w", g=2)[:, :, off:512],
                            s_ps[:].rearrange("p (g w) -> p g w", g=2)[:, :, off:512],
                            mybir.ActivationFunctionType.Exp,
                            scale=SCALE,
                        )
                        if 4 * j <= i:
                            for h in range(2):
                                nc.vector.tensor_tensor(
                                    pt[:, 512 * h + off : 512 * h + off + 128],
                                    pt[:, 512 * h + off : 512 * h + off + 128],
                                    trimask[:],
                                    op=mybir.AluOpType.mult,
                                )
                        for h in range(2):
                            nc.tensor.matmul(
                                o_ps[h][0:65, off:512],
                                v_sb[s][i][:, (D + 1) * h : (D + 1) * (h + 1)],
                                pt[:, 512 * h + off : 512 * (h + 1)],
                                start=(i == 0), stop=(i == ilast),
                            )
                    # rowsums, reciprocal, fused divide+evict for chunk j
                    for h in range(2):
                        idx = 2 * j + h
                        nc.vector.tensor_copy(
                            ra[0:1, 512 * idx : 512 * (idx + 1)],
                            o_ps[h][64:65, :],
                        )
                    rs = ra[0:1, 1024 * j : 1024 * j + 1024]
                    nc.vector.reciprocal_approx_fast(rs, rs)
                    rb = smp.tile([128, 1024], f32, name="rb", tag="rb", bufs=2)
                    nc.gpsimd.partition_broadcast(rb[:], rs)
                    for h in range(2):
                        nc.vector.tensor_tensor(
                            ot[64 * h : 64 * h + 64, 512 * j : 512 * (j + 1)],
                            o_ps[h][0:64, :],
                            rb[64 * h : 64 * h + 64, 512 * h : 512 * (h + 1)],
                            op=mybir.AluOpType.mult,
                        )
                    nc.sync.dma_start(a2a_in[4 * s + j],
                                       ot[:, 512 * j : 512 * (j + 1)])

            # slot-0 qkv runs standalone (PE-dense)
            for _ in qkv_steps(0):
                pass
            # slot-0 attention; slot-1 qkv runs standalone after it (dense
            # interleave was observed to deepen the HAM duty-cycle throttle)
            emit_attn(0, iter(()), 0, ps_o, 1)
            for _ in qkv_steps(1):
                pass

            # slot-1: free the qkv psum banks, double-buffer o_ps so the
            # divide chain at chunk boundaries overlaps the next chunk's PV
            attn_psum_o.__exit__(None, None, None)
            qkv_psum.__exit__(None, None, None)
            attn_psum_o2 = tc.tile_pool(name="ps_o2", bufs=2, space="PSUM")
            ps_o2 = attn_psum_o2.__enter__()
            emit_attn(1, iter(()), 0, ps_o2, 2)

            # ---- exchange: AllToAll over all 8 cores ----
            nc.gpsimd.collective_compute(
                "AllToAll", mybir.AluOpType.bypass,
                replica_groups=[list(range(NCORES))],
                ins=[a2a_in[:].opt()], outs=[a2a_out[:].opt()],
            )

            attn_psum_o2.__exit__(None, None, None)
            attn_psum_s.__exit__(None, None, None)

            # ---- output projection on own 512-row shard --------------------
            proj_psum = tc.tile_pool(name="psy", bufs=3, space="PSUM")
            psy = proj_psum.__enter__()
            yT_sb = []
            for k in range(KT):
                yt = projp.tile([128, 512], bf16, name=f"yT{k}")
                nc.sync.dma_start(yt[:], a2a_out[k])
                yT_sb.append(yt)
            for m in range(SHARD // 128):
                ysb = ysbp.tile([128, C], f32, name="ysb", tag="ysb")
                for n in range(2):
                    ps = psy.tile([128, 512], f32, name="ps_y", tag="ps_y")
                    for k in range(KT):
                        nc.tensor.matmul(
                            ps[:],
                            yT_sb[k][:, 128 * m : 128 * (m + 1)],
                            wp_sb[:, C * k + 512 * n : C * k + 512 * (n + 1)],
                            start=(k == 0), stop=(k == KT - 1),
                        )
                    nc.vector.tensor_tensor(
                        ysb[:, 512 * n : 512 * (n + 1)],
                        ps[:],
                        bpb[:, 512 * n : 512 * (n + 1)],
                        op=mybir.AluOpType.add,
                    )
                nc.sync.dma_start(y[128 * m : 128 * (m + 1), :], ysb[:])
            proj_psum.__exit__(None, None, None)

    nc.compile()
    return nc


def _get_nc():
    if "nc" not in _CACHE:
        _CACHE["nc"] = _build_nc()
    return _CACHE["nc"]


def kernel(x, W_attn, b_attn, W_proj, b_proj, _trace=False):
    global LAST_EXEC_NS
    from concourse.bass_utils import run_bass_kernel_spmd
    bf16 = np.float16

    x = np.asarray(x, np.float32)
    W_attn = np.asarray(W_attn, np.float32)
    b_attn = np.asarray(b_attn, np.float32)
    W_proj = np.asarray(W_proj, np.float32)
    b_proj = np.asarray(b_proj, np.float32)

    def pmajor(w):  # [C, M] -> [128, KT*M]
        m = w.shape[1]
        return np.ascontiguousarray(
            w.reshape(KT, 128, m).transpose(1, 0, 2).reshape(128, KT * m)
        ).astype(bf16)

    xT = np.transpose(x, (0, 2, 1))  # [B, C, T]
    xT16 = np.ascontiguousarray(
        xT.reshape(B, KT, 128, T).transpose(0, 2, 1, 3).reshape(B, 128, KT * T)
    ).astype(bf16)
    bp_h = np.ascontiguousarray(b_proj).reshape(1, C)

    wp16 = pmajor(W_proj)

    in_maps = []
    for c in range(NCORES):
        s_ = slice(CW * c, CW * (c + 1))
        in_maps.append({
            "xT": xT16,
            "wq": pmajor(W_attn[:, s_]),
            "wk": pmajor(W_attn[:, C:][:, s_]),
            "wv": pmajor(W_attn[:, 2 * C:][:, s_]),
            "wp": wp16,
            "bq": np.ascontiguousarray(b_attn[s_]).reshape(CW, 1),
            "bk": np.ascontiguousarray(b_attn[C:][s_]).reshape(CW, 1),
            "bv": np.ascontiguousarray(b_attn[2 * C:][s_]).reshape(CW, 1),
            "bp": bp_h,
        })

    nc = _get_nc()
    res = run_bass_kernel_spmd(nc, in_maps, list(range(NCORES)), trace=_trace)
    LAST_EXEC_NS = res.exec_time_ns

    out = np.empty((B, T, C), np.float32)
    for c in range(NCORES):
        out[c // 4, 512 * (c % 4) : 512 * (c % 4 + 1), :] = res.results[c]["y"]
    return out
